# revision 26
# baseline (speedup 1.0000x reference)
"""Causal ReLU-attention (no softmax) fused kernel for TRN2, 8 NeuronCores.

Reference computation (B=2, T=2048, C=1024, H=16, D=64):
    qkv = x @ W.T + b ; q,k,v split; per (b,h): y = relu(tril(q k^T / sqrt(D))) @ v

Sharding: core c handles batch b = c//4 and heads 4*(c%4) .. 4*(c%4)+3.
Each core is fully independent (no collectives).

All-bf16 datapath (inputs cast host-side; PSUM accumulation stays fp32;
measured end-to-end rel err ~5.9e-3 vs the 2e-2 gate):
  setup:   ACT activation-table pre-warm + PE p-state warm-up matmuls run
           during the DMA lead-in; tiny bias DMAs issue before the big
           xt/wt streams so evacuations never wait on bulk traffic.
  phase 1: qT/kT [256,2048] head-major (scale folded into Wq), v [2048,256]
           from xT bf16 [1024,2048] and wT bf16 [1024,768]; DMAs issued in
           matmul consumption order so the PE never waits on HBM; k PSUM at
           single-bank granularity with DVE/ACT evacs; v reuses the k banks
           piecewise (high banks first so phase-2 psum tiles are freed
           earliest).
  phase 2: per head-PAIR hp (heads 2hp,2hp+1 at SBUF partitions 0:64/64:128),
           per query-chunk qc (512 queries), per key block kb (128 keys):
           the two heads' ST matmuls use disjoint PE row tiles (64-row
           tile_position) and execute CONCURRENTLY; ReLU-evac to bf16 SBUF
           alternates ACT/DVE, trimmed to the causal query range; triangular
           masks on diag 128x128 tiles via affine_select on GpSimd; AV
           accumulates yT [128 = 2 heads x 64, 512] in one PSUM bank per
           pair (the two AV column groups partially overlap on the PE).
           The two head-pairs' task streams are interleaved batch-3 so the
           PE always has independent work while evacs are in flight. The
           final query chunk's output flushes in three pieces as the
           diagonal AVs finalize columns (non-final output DMAs issue from
           the Pool queue to keep the Sync queue off the kernel tail).
Output per core: yT [256, 2048] bf16; host upcasts/transposes into y[b, :, 256g:256g+256].
"""

import numpy as np

N_EMBD = 1024
N_HEAD = 16
HEAD_DIM = 64
B, T, C = 2, 2048, N_EMBD
NCORES = 8
P = 128
KC = C // P  # 8 contraction chunks
NQC = T // 512  # 4 query chunks

_NC_CACHE = {}


def _build_bass():
    import concourse.bass as bass
    from concourse import bacc, mybir
    from concourse.tile import TileContext

    f32 = mybir.dt.float32
    bf16 = mybir.dt.bfloat16

    nc = bacc.Bacc(None, target_bir_lowering=False)
    xt = nc.declare_dram_parameter("xt", [C, T], bf16, isOutput=False)
    wt = nc.declare_dram_parameter("wt", [C, 768], bf16, isOutput=False)
    bcol = nc.declare_dram_parameter("bcol", [512], f32, isOutput=False)
    bv = nc.declare_dram_parameter("bv", [256], f32, isOutput=False)
    out = nc.declare_dram_parameter("out", [256, T], bf16, isOutput=True)

    xt_r = xt[:, :].rearrange("(c p) t -> c p t", p=P)
    wt_r = wt[:, :].rearrange("(c p) o -> c p o", p=P)
    bv_ap = bv[:]
    HT = T // 2

    with TileContext(nc) as tc:
        with (
            tc.tile_pool(name="const", bufs=1) as const_pool,
            tc.tile_pool(name="qkv", bufs=1) as qkv_pool,
            tc.tile_pool(name="stsb", bufs=12) as stsb_pool,
            tc.tile_pool(name="ysb", bufs=6) as ysb_pool,
        ):
            # ACT act-table pre-warm: first ACTIVATE triggers a ~1.3us
            # ACT_TABLE_LOAD; run it here so it overlaps the DMA lead-in.
            warm = const_pool.tile([1, 8], f32, name="warm")
            nc.vector.memset(warm, 0.0)
            nc.scalar.activation(
                out=warm, in_=warm, func=mybir.ActivationFunctionType.Relu
            )

            xt_h = [
                const_pool.tile([P, KC, HT], bf16, name=f"xt_h{h}") for h in range(2)
            ]
            wt_sb = const_pool.tile([P, KC, 768], bf16)
            # Tiny bias DMAs FIRST: the q/k/v evacs read them, and a DMA
            # issued after the big xt stream would gate the first evacs
            # behind megabytes of input traffic (observed 3.6us PE stall).
            bqk_sb = const_pool.tile([P, 4], f32)
            nc.sync.dma_start(out=bqk_sb, in_=bcol[:].rearrange("(m p) -> p m", p=P))
            bv_rep = const_pool.tile([P, 256], f32)
            bv_bcast = bass.AP(
                tensor=bv_ap.tensor, offset=bv_ap.offset, ap=[[0, P], [1, 256]]
            )
            nc.sync.dma_start(out=bv_rep, in_=bv_bcast)
            # DMA order == phase-1 consumption order: (wt[c], xt_h0[c]) pairs,
            # then xt_h1 chunks.
            # chunk 0 split in halves so the first q/k matmuls (which only
            # need wt cols 0:512 and xt t 0:512) start ~1us earlier
            nc.sync.dma_start(out=wt_sb[:, 0, 0:512], in_=wt_r[0][:, 0:512])
            nc.sync.dma_start(out=xt_h[0][:, 0, 0:512], in_=xt_r[0][:, 0:512])
            nc.sync.dma_start(out=wt_sb[:, 0, 512:768], in_=wt_r[0][:, 512:768])
            nc.sync.dma_start(out=xt_h[0][:, 0, 512:HT], in_=xt_r[0][:, 512:HT])
            for c in range(1, KC):
                nc.sync.dma_start(out=wt_sb[:, c, :], in_=wt_r[c])
                nc.sync.dma_start(out=xt_h[0][:, c, :], in_=xt_r[c][:, 0:HT])
            for c in range(KC):
                nc.sync.dma_start(out=xt_h[1][:, c, :], in_=xt_r[c][:, HT:T])

            # PE p-state warm-up: the clock ramps 0.65->1.2->2.4 GHz over
            # ~3us of continuous PE activity. Dummy matmuls on a zeroed
            # scratch tile keep the array busy during the DMA lead-in so the
            # first real matmuls run at full clock.
            pe_warm = const_pool.tile([P, 512], bf16, name="pe_warm")
            nc.vector.memset(pe_warm, 0.0)
            with tc.tile_pool(name="psw", bufs=1, space="PSUM") as psw_pool:
                psw = psw_pool.tile([P, 512], f32, name="psw")
                for _ in range(7):
                    nc.tensor.matmul(
                        psw, pe_warm[:, 0:128], pe_warm, start=True, stop=True
                    )

            q_sb = qkv_pool.tile([P, 2, T], bf16)
            k_sb = qkv_pool.tile([P, 2, T], bf16)
            v_sb = qkv_pool.tile([P, T // P, 256], bf16)

            # ---- phase 1, by T-halves ----
            # q: 2 x [128,1024] psum (4 banks), ACT evac (bias via activation).
            # k: 4 x [128,512] psum (4 banks), DVE/Pool evacs; v reuses the
            # k banks piecewise so its matmuls start as soon as the first k
            # piece is evacuated.
            with tc.tile_pool(name="psum1", bufs=1, space="PSUM") as psum1:
                for half in range(2):
                    t0c = half * HT
                    pq = [
                        psum1.tile([P, HT], f32, tag=f"psA{m}", name=f"pq{m}")
                        for m in range(2)
                    ]
                    pk = [
                        psum1.tile([P, 512], f32, tag=f"psB{i}", name=f"pk{i}")
                        for i in range(4)
                    ]
                    for c in range(KC):
                        for m in range(2):
                            for n in range(2):
                                nc.tensor.matmul(
                                    pk[2 * m + n],
                                    wt_sb[:, c, 256 + m * P : 256 + (m + 1) * P],
                                    xt_h[half][:, c, n * 512 : (n + 1) * 512],
                                    start=(c == 0),
                                    stop=(c == KC - 1),
                                )
                        for m in range(2):
                            for n in range(2):
                                nc.tensor.matmul(
                                    pq[m][:, n * 512 : (n + 1) * 512],
                                    wt_sb[:, c, m * P : (m + 1) * P],
                                    xt_h[half][:, c, n * 512 : (n + 1) * 512],
                                    start=(c == 0),
                                    stop=(c == KC - 1),
                                )
                    # k evacs: one per bank, DVE/ACT alternating (GpSimd
                    # cannot read PSUM); q evacs follow on ACT.
                    for idx, i in enumerate([3, 2, 1, 0]):
                        m, n = i // 2, i % 2
                        if idx % 2 == 0:
                            nc.vector.tensor_scalar_add(
                                k_sb[:, m, t0c + n * 512 : t0c + (n + 1) * 512],
                                pk[i],
                                bqk_sb[:, 2 + m : 3 + m],
                            )
                        else:
                            nc.scalar.activation(
                                out=k_sb[:, m, t0c + n * 512 : t0c + (n + 1) * 512],
                                in_=pk[i],
                                func=mybir.ActivationFunctionType.Identity,
                                bias=bqk_sb[:, 2 + m : 3 + m],
                                scale=1.0,
                            )
                    for m in range(2):
                        nc.scalar.activation(
                            out=q_sb[:, m, t0c : t0c + HT],
                            in_=pq[m],
                            func=mybir.ActivationFunctionType.Identity,
                            bias=bqk_sb[:, m : m + 1],
                            scale=1.0,
                        )
                    # v: 4 pieces of 2 t-blocks, each in a freed k bank.
                    bv_in = bass.AP(
                        tensor=bv_rep.tensor,
                        offset=bv_rep.offset,
                        ap=[bv_rep.ap[0], [0, 2], [1, 256]],
                    )
                    for piece in range(4):
                        # allocate from high tags first so the banks phase-2's
                        # first psum tiles land on are evacuated earliest
                        pv = psum1.tile(
                            [P, 512], f32, tag=f"psB{3 - piece}", name=f"pv{piece}"
                        )
                        for tl in range(2):
                            tbl = piece * 2 + tl
                            for c in range(KC):
                                nc.tensor.matmul(
                                    pv[:, tl * 256 : (tl + 1) * 256],
                                    xt_h[half][:, c, tbl * P : (tbl + 1) * P],
                                    wt_sb[:, c, 512:768],
                                    start=(c == 0),
                                    stop=(c == KC - 1),
                                )
                        v_dst = v_sb[
                            :, half * 8 + piece * 2 : half * 8 + (piece + 1) * 2, :
                        ]
                        nc.vector.scalar_tensor_tensor(
                            out=v_dst,
                            in0=pv.rearrange("p (a b) -> p a b", a=2),
                            scalar=0.0,
                            in1=bv_in,
                            op0=mybir.AluOpType.add,
                            op1=mybir.AluOpType.add,
                        )

            # ---- phase 2: attention, interleaved head-pair task streams ----
            with (
                tc.tile_pool(name="pst", bufs=3, space="PSUM") as pst_pool,
                tc.tile_pool(name="py", bufs=1, space="PSUM") as py_pool,
            ):
                yps = [None, None]
                evac_ctr = 0
                out_ctr = 0

                def relu_evac(dst, src):
                    nonlocal evac_ctr
                    r = evac_ctr % 2
                    evac_ctr += 1
                    if r == 0:
                        nc.scalar.activation(
                            out=dst, in_=src,
                            func=mybir.ActivationFunctionType.Relu,
                        )
                    else:
                        nc.vector.tensor_scalar_max(dst, src, 0.0)

                def st_block(hp, qc, kb):
                    d = kb - 4 * qc
                    c0 = P * d if d > 0 else 0
                    stps = pst_pool.tile([P, 2, 512], f32, tag="stps", name="stps")
                    stsb = stsb_pool.tile([P, 2, 512], bf16, tag="stsb", name="stsb")
                    for hh in range(2):
                        off = hh * 64
                        nc.tensor.matmul(
                            stps[:, hh, c0:512],
                            k_sb[off : off + 64, hp, kb * P : (kb + 1) * P],
                            q_sb[off : off + 64, hp, qc * 512 + c0 : (qc + 1) * 512],
                            start=True,
                            stop=True,
                        )
                    relu_evac(stsb[:, :, c0:512], stps[:, :, c0:512])
                    if d >= 0:
                        # triangular mask on the diag 128-col tile, both heads
                        # in one strided op on GpSimd (SBUF in-place)
                        base = stsb[:, 0, P * d : P * d + P]
                        tri = bass.AP(
                            tensor=base.tensor,
                            offset=base.offset,
                            ap=[base.ap[0], [512, 2], [1, P]],
                        )
                        nc.gpsimd.affine_select(
                            out=tri,
                            in_=tri,
                            pattern=[[0, 2], [1, P]],
                            compare_op=mybir.AluOpType.is_ge,
                            fill=0.0,
                            base=0,
                            channel_multiplier=-1,
                        )
                    return stsb

                def av_block(hp, qc, kb, stsb, first, last):
                    d = kb - 4 * qc
                    c0 = P * d if d > 0 else 0
                    if first:
                        # lazy per-(hp,qc) allocation: the bank-reuse WAR wait
                        # lands here, SKEW tasks into the stream, not ahead of
                        # the first ST
                        yps[hp] = py_pool.tile(
                            [P, 512], f32, tag=f"yps{hp}", name=f"yps{hp}"
                        )
                    for hh in range(2):
                        h = 2 * hp + hh
                        nc.tensor.matmul(
                            yps[hp][hh * 64 : (hh + 1) * 64, c0:512],
                            v_sb[:, kb, h * 64 : (h + 1) * 64],
                            stsb[:, hh, c0:512],
                            start=first,
                            stop=last,
                        )

                def emit_out(hp, qc, lo, hi):
                    nonlocal out_ctr
                    ysb = ysb_pool.tile([P, 512], bf16, tag="ysb", name="ysb")
                    r = out_ctr % 2
                    out_ctr += 1
                    if r == 0:
                        nc.scalar.copy(ysb[:, lo:hi], yps[hp][:, lo:hi])
                    else:
                        nc.vector.tensor_copy(ysb[:, lo:hi], yps[hp][:, lo:hi])
                    # Non-final output DMAs issue from the Pool queue so
                    # the Sync queue's ~600ns per-issue cost never serializes
                    # into the kernel tail; the very last piece uses Sync.
                    eng = nc.sync if (qc == NQC - 1 and lo > 0) else nc.gpsimd
                    eng.dma_start(
                        out=out[hp * P : (hp + 1) * P, qc * 512 + lo : qc * 512 + hi],
                        in_=ysb[:, lo:hi],
                    )

                # Unified task list: both head-pairs at the same qc, kb
                # ascending, hp innermost so adjacent tasks are independent.
                tasks = []
                for qc in range(NQC):
                    nblocks = 4 * (qc + 1)
                    for kb in range(nblocks):
                        for hp in range(2):
                            tasks.append((hp, qc, kb, kb == 0, kb == nblocks - 1))

                def after_av(hp, qc, kb, last):
                    # output evac/DMA; the final query chunk flushes in three
                    # pieces as its diagonal AVs finalize columns, so the last
                    # DMA is small and its flight doesn't stretch the tail.
                    if qc == NQC - 1:
                        if kb == 4 * qc + 1:
                            emit_out(hp, qc, 0, 128)
                        elif kb == 4 * qc + 2:
                            emit_out(hp, qc, 128, 384)
                        elif last:
                            emit_out(hp, qc, 384, 512)
                    elif last:
                        emit_out(hp, qc, 0, 512)

                # Software pipeline, batch-2: emit two tasks' AVs then two
                # tasks' STs so the PE switches between the 64-row ST tile
                # config and the 128-row AV config half as often, and AVs are
                # never head-of-line-blocked behind a psum-WAR-stalled ST.
                SKEW = 9
                pending = []

                def drain_avs(nmax):
                    done = 0
                    while pending and len(pending) > SKEW - 3 and done < nmax:
                        (phw, pqc, pkb, pfirst, plast), pst = pending.pop(0)
                        av_block(phw, pqc, pkb, pst, pfirst, plast)
                        after_av(phw, pqc, pkb, plast)
                        done += 1

                for i in range(0, len(tasks), 3):
                    batch = tasks[i : i + 3]
                    if len(pending) >= SKEW:
                        drain_avs(3)
                    for t in batch:
                        hp, qc, kb, first, last = t
                        pending.append((t, st_block(hp, qc, kb)))
                for (phw, pqc, pkb, pfirst, plast), pst in pending:
                    av_block(phw, pqc, pkb, pst, pfirst, plast)
                    after_av(phw, pqc, pkb, plast)

    nc.compile()
    return nc


def _get_nc():
    if "nc" not in _NC_CACHE:
        _NC_CACHE["nc"] = _build_bass()
    return _NC_CACHE["nc"]


def make_in_maps(x: np.ndarray, W: np.ndarray, b: np.ndarray):
    import ml_dtypes

    bf = ml_dtypes.bfloat16
    scale = np.float32(1.0 / np.sqrt(HEAD_DIM))
    xts = [np.ascontiguousarray(x[bb].T.astype(bf)) for bb in range(B)]
    in_maps = []
    for core in range(NCORES):
        bb, g = core // 4, core % 4
        o0 = g * 256
        wq = W[o0 : o0 + 256, :] * scale
        wk = W[C + o0 : C + o0 + 256, :]
        wv = W[2 * C + o0 : 2 * C + o0 + 256, :]
        wt = np.ascontiguousarray(
            np.concatenate([wq.T, wk.T, wv.T], axis=1).astype(bf)
        )
        bq = b[o0 : o0 + 256] * scale
        bk = b[C + o0 : C + o0 + 256]
        bvv = np.ascontiguousarray(b[2 * C + o0 : 2 * C + o0 + 256], dtype=np.float32)
        bcol = np.ascontiguousarray(np.concatenate([bq, bk]), dtype=np.float32)
        in_maps.append({"xt": xts[bb], "wt": wt, "bcol": bcol, "bv": bvv})
    return in_maps


def kernel(x: np.ndarray, W: np.ndarray, b: np.ndarray) -> np.ndarray:
    from concourse.bass_utils import run_bass_kernel_spmd

    x = np.asarray(x, dtype=np.float32)
    W = np.asarray(W, dtype=np.float32)
    b = np.asarray(b, dtype=np.float32)

    nc = _get_nc()
    in_maps = make_in_maps(x, W, b)
    res = run_bass_kernel_spmd(nc, in_maps, core_ids=list(range(NCORES)))

    y = np.empty((B, T, C), dtype=np.float32)
    for core in range(NCORES):
        bb, g = core // 4, core % 4
        y[bb, :, g * 256 : (g + 1) * 256] = (
            res.results[core]["out"].astype(np.float32).T
        )
    return y
